# revision 1
# baseline (speedup 1.0000x reference)
"""Trainium2 Bass kernel for nn_CanadarmJacob (centroidal-dynamics jacobian).

Pure data-parallel over 8 NeuronCores: core c processes samples
[c*256:(c+1)*256] of the n_samples axis (x 128 horizon = 32768 flat
samples/core).  Per core the work is split into NBLK blocks of 128
(partitions) x F (free) samples; every per-sample scalar channel lives as a
strided view into sample-major SBUF tiles, so each graph node is one
vector-engine instruction over 128*F samples.

The math is an algebraically reduced form of the reference (validated to
~1e-5 rel):
  RP = C - P ;  MC = m_i*C ;  U[a,dd,i] = RP[a]*(MC[dd] | m_i)
  G = suffix_j(U)  ->  G[a,d,j], R[a,j] (dd=3 row)
  r = sum_i MC / M_tot - (0,0,beta)
  T[a,j] = sum_d G[a,d,j]*J[d,j] ;  trG, rR, rJ
  u = trG - beta*R_z - rR ;  v = beta*J_z + rJ
  H_theta = (DCUM + u)*J - T + v*R
  J_tw = J_j x R_j
  H_s = K r r^T + diag(C1 - K|r|^2) ;  sInv = -adj(H_s)/det
  bot = sInv @ H_theta ;  top = -J_tw/M_tot + r x bot
"""

import os
import sys

for _p in ("/opt/trn_rl_repo", "/root/.axon_site/_ro/trn_rl_repo"):
    if os.path.isdir(_p) and _p not in sys.path:
        sys.path.append(_p)

import numpy as np

import concourse.bass as bass
import concourse.tile as tile
from concourse import bacc, mybir
from concourse.bass_utils import run_bass_kernel_spmd

# ----------------------------------------------------------------- constants
N_SAMPLES, N_HORIZON = 2048, 128
N_CORES = 8
P = 128
F = 32  # samples per partition row per block
SPC = N_SAMPLES // N_CORES * N_HORIZON  # samples per core = 32768
NBLK = SPC // (P * F)  # 8

BASE_MASS, EEF_MASS = 100000.0, 243.66
MASS = np.array([105.98, 105.98, 314.98, 279.2, 105.98, 105.98, 243.66], np.float32)
DIAGS = np.array(
    [
        [12.19, 12.19, 3.061],
        [12.19, 12.19, 3.061],
        [15.41, 2094.71, 2103.19],
        [9.522, 1966.28, 1966.28],
        [8.305, 3.061, 8.0386],
        [12.13, 12.13, 3.061],
        [9.336, 44.41, 44.41],
    ],
    np.float32,
)
I0DIAG = np.array([69585.02, 69585.02, 66666.664], np.float32)

M_MAN = float(MASS.sum())
M_TOT = M_MAN + BASE_MASS + EEF_MASS
K = BASE_MASS + EEF_MASS
BETA = 6.65 * (243.66 / (100000.0 + 243.66))
DCUM = np.stack([DIAGS[j:].sum(0) for j in range(7)], axis=1)  # [a][j]
C1 = DIAGS.sum(0) + I0DIAG  # [a]

DT = mybir.dt.float32
ADD = mybir.AluOpType.add
SUB = mybir.AluOpType.subtract
MUL = mybir.AluOpType.mult


def _const_array() -> np.ndarray:
    cst = np.zeros((P, 45), np.float32)
    cst[:, 0:21] = np.broadcast_to(MASS[None, :], (3, 7)).reshape(21)[None, :]
    cst[:, 21:42] = DCUM.reshape(21)[None, :]
    cst[:, 42:45] = C1[None, :]
    return cst


def build_nc():
    nc = bacc.Bacc("TRN2")

    com_in = nc.dram_tensor("com", [NBLK, P, F * 21], DT, kind="ExternalInput")
    lnk_in = nc.dram_tensor("lnk", [NBLK, P, F, 144], DT, kind="ExternalInput")
    jac_in = nc.dram_tensor("jac", [NBLK, P, F * 42], DT, kind="ExternalInput")
    cst_in = nc.dram_tensor("cst", [P, 45], DT, kind="ExternalInput")
    out_d = nc.dram_tensor("out", [NBLK, P, F * 42], DT, kind="ExternalOutput")

    V = nc.vector
    G_ = nc.gpsimd
    A_ = nc.scalar
    X = mybir.AxisListType.X

    with tile.TileContext(nc) as tc:
        with (
            tc.tile_pool(name="cstp", bufs=1) as cstp,
            tc.tile_pool(name="iocl", bufs=2) as iocl,
            tc.tile_pool(name="iojo", bufs=3) as iojo,
            tc.tile_pool(name="wk", bufs=3) as wk,
        ):
            cst = cstp.tile([P, 45], DT, tag="cst")
            nc.scalar.dma_start(cst[:], cst_in[:])
            massc = (
                cst[:, 0:21]
                .rearrange("p (a i) -> p a i", a=3, i=7)
                .unsqueeze(1)
                .broadcast_to([P, F, 3, 7])
            )
            dcum_v = (
                cst[:, 21:42]
                .rearrange("p (a j) -> p a j", a=3, j=7)
                .unsqueeze(1)
                .broadcast_to([P, F, 3, 7])
            )
            c1_v = cst[:, 42:45].unsqueeze(1).broadcast_to([P, F, 3])

            def rv(t, groups, **kw):
                return t[:].rearrange(f"p (f {groups}) -> p f {groups}", f=F, **kw)

            def front(b, E=None):
                """DMA + all Pool-engine work for block b.  Pool(b) depends
                only on block b's DMAs — it never waits on DVE."""
                st = {}
                E = E or G_
                comt = iocl.tile([P, F * 21], DT, tag="comt")
                lnkt = iocl.tile([P, F * 108], DT, tag="lnkt")
                jact = iojo.tile([P, F * 42], DT, tag="jact")
                nc.scalar.dma_start(comt[:], com_in[b])
                lnkv = lnkt[:].rearrange("p (f e) -> p f e", f=F, e=108)
                half = F // 2
                nc.sync.dma_start(lnkv[:, 0:half, :], lnk_in[b, :, 0:half, 0:108])
                nc.scalar.dma_start(lnkv[:, half:F, :], lnk_in[b, :, half:F, 0:108])
                nc.scalar.dma_start(jact[:], jac_in[b])

                comv = rv(comt, "a i", a=3, i=7)
                lall = rv(lnkt, "a q i", a=3, q=4, i=9)
                posv = lall[:, :, 0:3, 3, 0:7]
                jacv = rv(jact, "d j", d=6, j=7)
                j3 = jacv[:, :, 0:3, :]
                st["jact"], st["j3"] = jact, j3

                mc = wk.tile([P, F * 21], DT, tag="mc")
                mcv = rv(mc, "a i", a=3, i=7)
                G_.tensor_mul(mcv, comv, massc)

                # rpre[a] = sum_i MC[a][i]  (unscaled; /M_tot folded downstream)
                y9 = wk.tile([P, F * 9], DT, tag="y9")
                y9v = rv(y9, "a c", a=3, c=3)
                G_.tensor_add(y9v, mcv[:, :, :, 0:3], mcv[:, :, :, 3:6])
                rt = wk.tile([P, F * 3], DT, tag="rt")
                rtv = rv(rt, "a", a=3)
                G_.tensor_add(rtv, y9v[:, :, :, 0], y9v[:, :, :, 1])
                G_.tensor_add(rtv, rtv, y9v[:, :, :, 2])
                G_.tensor_add(rtv, rtv, mcv[:, :, :, 6])
                st["rtv"] = rtv

                # rj'[j] = sum_a rpre[a] * J[a][j]
                r_bj = rtv.unsqueeze(3).broadcast_to([P, F, 3, 7])
                rjp = wk.tile([P, F * 21], DT, tag="rjp")
                rjpv = rv(rjp, "a j", a=3, j=7)
                G_.tensor_mul(rjpv, r_bj, j3)
                rj = wk.tile([P, F * 7], DT, tag="rj")
                rjv = rv(rj, "j", j=7)
                G_.tensor_add(rjv, rjpv[:, :, 0, :], rjpv[:, :, 1, :])
                G_.tensor_add(rjv, rjv, rjpv[:, :, 2, :])

                rp = wk.tile([P, F * 21], DT, tag="rp")
                rpv = rv(rp, "a i", a=3, i=7)
                E.tensor_sub(rpv, comv, posv)

                ut = wk.tile([P, F * 84], DT, tag="ut")
                utv = rv(ut, "a dd i", a=3, dd=4, i=7)
                for a in range(3):
                    rp_b = rpv[:, :, a : a + 1, :].broadcast_to([P, F, 3, 7])
                    E.tensor_mul(utv[:, :, a, 0:3, :], rp_b, mcv)
                E.tensor_mul(utv[:, :, :, 3, :], rpv, massc)

                # suffix sums over last index (j = 5..0), in place on ut
                gtv = rv(ut, "a dd j", a=3, dd=4, j=7)
                for j in range(5, -1, -1):
                    E.tensor_add(
                        gtv[:, :, :, :, j], gtv[:, :, :, :, j], gtv[:, :, :, :, j + 1]
                    )
                gd = gtv[:, :, :, 0:3, :]
                rsuf = gtv[:, :, :, 3, :]
                st["gd"], st["rsuf"] = gd, rsuf

                trg = wk.tile([P, F * 7], DT, tag="trg")
                trgv = rv(trg, "j", j=7)
                E.tensor_add(trgv, gd[:, :, 0, 0, :], gd[:, :, 1, 1, :])
                E.tensor_add(trgv, trgv, gd[:, :, 2, 2, :])
                st["trgv"] = trgv

                # VR' = rj'_b * R
                vrv = rjpv  # rjp dead after tree; reuse for VR'
                v_b = rjv.unsqueeze(2).broadcast_to([P, F, 3, 7])
                G_.tensor_mul(vrv, v_b, rsuf)
                st["vrv"] = vrv



                st["mcv"] = mcv  # mc dead after rpre; reuse for H_theta
                st["rpv"] = rpv  # rp dead after U-products; reuse for RRp'/T
                return st

            def back(st, b, EJ=None):
                """All DVE work for block b (+ ACT bits + output DMA)."""
                EJ = EJ or V
                j3, gd, rsuf = st["j3"], st["gd"], st["rsuf"]
                rtv = st["rtv"]

                # rs = rpre/M_tot - (0,0,beta)
                rs = wk.tile([P, F * 3], DT, tag="rs")
                rsv = rv(rs, "a", a=3)
                A_.mul(rsv, st["rtv"], 1.0 / M_TOT)
                V.tensor_scalar_add(rsv[:, :, 2:3], rsv[:, :, 2:3], -BETA)

                # RRp' = rpre_b * R ; rr' tree ; u = trG - rr'/M_tot
                rrpv = st["rpv"]  # reuse rp tile
                r_bj = rtv.unsqueeze(3).broadcast_to([P, F, 3, 7])
                V.tensor_mul(rrpv, r_bj, rsuf)
                rr = wk.tile([P, F * 7], DT, tag="rr")
                rrv = rv(rr, "j", j=7)
                V.tensor_add(rrv, rrpv[:, :, 0, :], rrpv[:, :, 1, :])
                V.tensor_add(rrv, rrv, rrpv[:, :, 2, :])
                ut7 = wk.tile([P, F * 7], DT, tag="ut7")
                ut7v = rv(ut7, "j", j=7)
                V.scalar_tensor_tensor(ut7v, rrv, -1.0 / M_TOT, st["trgv"], MUL, ADD)

                a1 = wk.tile([P, F * 21], DT, tag="a1")
                a1v = rv(a1, "a j", a=3, j=7)
                u_b = ut7v.unsqueeze(2).broadcast_to([P, F, 3, 7])
                V.tensor_add(a1v, u_b, dcum_v)

                # PROD1 overwrites gd in place; T tree into rrp (rp tile, now dead)
                for a in range(3):
                    V.tensor_mul(gd[:, :, a, :, :], gd[:, :, a, :, :], j3)
                ttv = rrpv
                V.tensor_add(ttv, gd[:, :, :, 0, :], gd[:, :, :, 1, :])
                V.tensor_add(ttv, ttv, gd[:, :, :, 2, :])

                # H_theta = A1*J - T + VR'/M_tot   (into mc tile)
                hthv = st["mcv"]
                V.tensor_mul(hthv, a1v, j3)
                V.tensor_sub(hthv, hthv, ttv)
                V.scalar_tensor_tensor(hthv, st["vrv"], 1.0 / M_TOT, hthv, MUL, ADD)

                # J_tw[a] = J[a1]*R[a2] - J[a2]*R[a1]
                jtw = wk.tile([P, F * 21], DT, tag="jtw")
                jtwv = rv(jtw, "a j", a=3, j=7)
                cx1 = wk.tile([P, F * 21], DT, tag="cx1")
                cx1v = rv(cx1, "a j", a=3, j=7)
                for a in range(3):
                    a1_, a2_ = (a + 1) % 3, (a + 2) % 3
                    EJ.tensor_mul(jtwv[:, :, a, :], j3[:, :, a1_, :], rsuf[:, :, a2_, :])
                    EJ.tensor_mul(cx1v[:, :, a, :], j3[:, :, a2_, :], rsuf[:, :, a1_, :])
                EJ.tensor_sub(jtwv, jtwv, cx1v)

                # --------------------------------------- H_s + inverse
                rk = wk.tile([P, F * 3], DT, tag="rk")
                rkv = rv(rk, "a", a=3)
                A_.mul(rkv, rsv, K)

                hs = wk.tile([P, F * 6], DT, tag="hs")
                hsv = rv(hs, "k", k=6)
                rk0 = rkv[:, :, 0:1].broadcast_to([P, F, 3])
                rk1 = rkv[:, :, 1:2].broadcast_to([P, F, 2])
                V.tensor_mul(hsv[:, :, 0:3], rk0, rsv[:, :, 0:3])
                V.tensor_mul(hsv[:, :, 3:5], rk1, rsv[:, :, 1:3])
                V.tensor_mul(hsv[:, :, 5], rkv[:, :, 2], rsv[:, :, 2])
                kr2 = wk.tile([P, F], DT, tag="kr2")
                kr2v = kr2[:]
                V.tensor_add(kr2v, hsv[:, :, 0], hsv[:, :, 3])
                V.tensor_add(kr2v, kr2v, hsv[:, :, 5])
                t3 = wk.tile([P, F * 3], DT, tag="t3")
                t3v = rv(t3, "a", a=3)
                V.tensor_tensor(
                    t3v, c1_v, kr2v.unsqueeze(2).broadcast_to([P, F, 3]), SUB
                )
                V.tensor_add(hsv[:, :, 0:4:3], hsv[:, :, 0:4:3], t3v[:, :, 0:2])
                V.tensor_add(hsv[:, :, 5], hsv[:, :, 5], t3v[:, :, 2])

                cof = wk.tile([P, F * 6], DT, tag="cof")
                cofv = rv(cof, "k", k=6)
                cp = [(3, 5), (2, 4), (1, 4), (0, 5), (1, 2), (0, 3)]
                cq = [(4, 4), (1, 5), (2, 3), (2, 2), (0, 4), (1, 1)]
                tmp6 = wk.tile([P, F * 6], DT, tag="tmp6")
                tmp6v = rv(tmp6, "k", k=6)
                for k in range(6):
                    V.tensor_mul(cofv[:, :, k], hsv[:, :, cp[k][0]], hsv[:, :, cp[k][1]])
                    V.tensor_mul(tmp6v[:, :, k], hsv[:, :, cq[k][0]], hsv[:, :, cq[k][1]])
                V.tensor_sub(cofv, cofv, tmp6v)

                detp = wk.tile([P, F * 3], DT, tag="detp")
                detpv = rv(detp, "a", a=3)
                V.tensor_mul(detpv, hsv[:, :, 0:3], cofv[:, :, 0:3])
                det = wk.tile([P, F], DT, tag="det")
                detv = det[:]
                V.tensor_add(detv, detpv[:, :, 0], detpv[:, :, 1])
                V.tensor_add(detv, detv, detpv[:, :, 2])
                rec = wk.tile([P, F], DT, tag="rec")
                recv = rec[:]
                V.reciprocal(recv, detv)
                sinv = wk.tile([P, F * 6], DT, tag="sinv")
                sinvv = rv(sinv, "k", k=6)
                V.scalar_tensor_tensor(
                    sinvv,
                    cofv,
                    -1.0,
                    recv.unsqueeze(2).broadcast_to([P, F, 6]),
                    MUL,
                    MUL,
                )

                # --------------------------------------- outputs
                outt = iojo.tile([P, F * 42], DT, tag="outt")
                outv = rv(outt, "row j", row=6, j=7)

                # bot[a] = sum_d sInv[rows[a][d]] * Hth[d]; d=0 column of the
                # symmetric inverse is sinv[0:3] (contiguous) -> one 21F op
                bta = wk.tile([P, F * 21], DT, tag="bta")
                btav = rv(bta, "a j", a=3, j=7)
                btb = wk.tile([P, F * 21], DT, tag="btb")
                btbv = rv(btb, "a j", a=3, j=7)
                s_d0 = sinvv.unsqueeze(3)[:, :, 0:3, :].broadcast_to([P, F, 3, 7])
                h_d0 = hthv[:, :, 0:1, :].broadcast_to([P, F, 3, 7])
                V.tensor_mul(btav, s_d0, h_d0)
                for a, k in enumerate((1, 3, 4)):  # d=1 column: rows (1,3,4)
                    sk = sinvv[:, :, k].unsqueeze(2).broadcast_to([P, F, 7])
                    V.tensor_mul(btbv[:, :, a, :], sk, hthv[:, :, 1, :])
                V.tensor_add(btav, btav, btbv)
                for a, k in enumerate((2, 4, 5)):  # d=2 column: rows (2,4,5)
                    sk = sinvv[:, :, k].unsqueeze(2).broadcast_to([P, F, 7])
                    V.tensor_mul(btbv[:, :, a, :], sk, hthv[:, :, 2, :])
                V.tensor_add(outv[:, :, 3:6, :], btav, btbv)

                # top[a] = -Jtw[a]/M_tot + (r x bot)[a]
                ctbv = a1v  # a1 dead after hth mul
                ctcv = cx1v  # cx1 dead after jtw sub
                for a in range(3):
                    a1_, a2_ = (a + 1) % 3, (a + 2) % 3
                    r1 = rsv[:, :, a1_].unsqueeze(2).broadcast_to([P, F, 7])
                    r2 = rsv[:, :, a2_].unsqueeze(2).broadcast_to([P, F, 7])
                    EJ.tensor_mul(ctbv[:, :, a, :], r1, outv[:, :, 3 + a2_, :])
                    EJ.tensor_mul(ctcv[:, :, a, :], r2, outv[:, :, 3 + a1_, :])
                V.tensor_sub(ctbv, ctbv, ctcv)
                V.scalar_tensor_tensor(
                    outv[:, :, 0:3, :], jtwv, -1.0 / M_TOT, ctbv, MUL, ADD
                )

                nc.scalar.dma_start(out_d[b], outt[:])

            st_prev = None
            for b in range(NBLK):
                st = front(b, E=V if b == 0 else None)
                if st_prev is not None:
                    back(st_prev, b - 1, EJ=G_ if b - 1 >= NBLK - 2 else None)
                st_prev = st
            back(st_prev, NBLK - 1, EJ=G_)

    nc.compile()
    return nc


_NC_CACHE = None


def _get_nc():
    global _NC_CACHE
    if _NC_CACHE is None:
        _NC_CACHE = build_nc()
    return _NC_CACHE


def _shard_inputs(com_list, link_pose_list, jacobian):
    com = np.ascontiguousarray(np.asarray(com_list, np.float32))
    lnk = np.ascontiguousarray(np.asarray(link_pose_list, np.float32))
    jac = np.ascontiguousarray(np.asarray(jacobian, np.float32))
    npc = N_SAMPLES // N_CORES
    cst = _const_array()
    in_maps = []
    for c in range(N_CORES):
        sl = slice(c * npc, (c + 1) * npc)
        in_maps.append(
            {
                "com": com[sl].reshape(NBLK, P, F * 21),
                "lnk": lnk[sl].reshape(NBLK, P, F, 144),
                "jac": jac[sl].reshape(NBLK, P, F * 42),
                "cst": cst,
            }
        )
    return in_maps


def _gather(results):
    outs = [r["out"].reshape(-1, 6, 7) for r in results]
    full = np.concatenate(outs, axis=0)
    return full.reshape(N_SAMPLES, N_HORIZON, 6, 7).astype(np.float32)


def run(com_list, link_pose_list, jacobian, trace=False):
    nc = _get_nc()
    in_maps = _shard_inputs(com_list, link_pose_list, jacobian)
    res = run_bass_kernel_spmd(nc, in_maps, list(range(N_CORES)), trace=trace)
    return _gather(res.results), res


def kernel(com_list, link_pose_list, jacobian):
    out, _ = run(com_list, link_pose_list, jacobian)
    return out



# revision 10
# speedup vs baseline: 1.0169x; 1.0169x over previous
"""Trainium2 Bass kernel for nn_CanadarmJacob (centroidal-dynamics jacobian).

Data-parallel over 8 NeuronCores; per core 32768 flat samples split into
NBLK=4 blocks of [P=128 partitions, F=64 free].  All per-sample quantities
live channel-major ([P, ch*F]) so every vector-op operand has a unit-stride
F-sized last dim -> DVE 2-byte fast modes apply.  Whole pipeline is bf16
scalar_tensor_tensor / tensor_scalar (InstTensorScalarPtr: 4x on DVE,
0.60-eff on Pool); ops are shaped so every access pattern canonicalizes to
<= 3 dims (stt verifier limit).  Only the 63 input floats/sample the
reference actually reads are shipped (com 21, link positions 21, jacobian
rows 0:3), packed host-side into one fused bf16 tensor.

Math (same validated algebra as the fp32 baseline):
  RP = C - P ;  MC = m_i*C ;  U[a,dd,i] = RP[a]*(MC[dd] | m_i)
  G = suffix_j(U) -> G[a,d,j], R[a,j] ;  rt = sum_i MC
  r = rt/M_tot - (0,0,beta) ;  T[a,j] = sum_d G[a,d,j]*J[d,j]
  trG ; rr = sum_a rt[a]R[a,j] ; rj = sum_a rt[a]J[a,j]
  u = trG - rr/M_tot ;  H_th = (DCUM + u)*J - T + (rj_b*R)/M_tot
  J_tw = J_j x R_j
H_s = K r r^T + diag(C1 - K|r|^2) = D + P_m with D = diag(C1) constant and
|P_m|/|D| <= ~4e-3, so first-order Neumann:  H_s^-1 ~= D^-1 - D^-1 P_m D^-1
  Y = Hth/C1 ;  s = sum_a r[a]Y[a,:] ;  q = |r|^2
  bot = w[a]*s - g[a]*Y ,  g = 1 + (K/C1[a]) q ,  w = (K/C1[a]) r[a]
  top = -J_tw/M_tot + r x bot
"""

import os
import sys

for _p in ("/opt/trn_rl_repo", "/root/.axon_site/_ro/trn_rl_repo"):
    if os.path.isdir(_p) and _p not in sys.path:
        sys.path.append(_p)

import numpy as np
import ml_dtypes

import concourse.bass as bass
import concourse.tile as tile
from concourse import bacc, mybir
from concourse.bass_utils import run_bass_kernel_spmd

# ----------------------------------------------------------------- constants
N_SAMPLES, N_HORIZON = 2048, 128
N_CORES = 8
P = 128
F = 64
SPC = N_SAMPLES // N_CORES * N_HORIZON  # 32768
NBLK = SPC // (P * F)  # 4

BASE_MASS, EEF_MASS = 100000.0, 243.66
MASS = np.array([105.98, 105.98, 314.98, 279.2, 105.98, 105.98, 243.66], np.float32)
DIAGS = np.array(
    [
        [12.19, 12.19, 3.061],
        [12.19, 12.19, 3.061],
        [15.41, 2094.71, 2103.19],
        [9.522, 1966.28, 1966.28],
        [8.305, 3.061, 8.0386],
        [12.13, 12.13, 3.061],
        [9.336, 44.41, 44.41],
    ],
    np.float32,
)
I0DIAG = np.array([69585.02, 69585.02, 66666.664], np.float32)

M_MAN = float(MASS.sum())
M_TOT = M_MAN + BASE_MASS + EEF_MASS
K = BASE_MASS + EEF_MASS
BETA = 6.65 * (243.66 / (100000.0 + 243.66))
DCUM = np.stack([DIAGS[j:].sum(0) for j in range(7)], axis=1)  # [a][j]
C1 = (DIAGS.sum(0) + I0DIAG).astype(np.float64)  # [a]

BF = mybir.dt.bfloat16
NPBF = ml_dtypes.bfloat16
ADD = mybir.AluOpType.add
MUL = mybir.AluOpType.mult

NCST = 42  # massc 21 | dcum 21


def _const_array() -> np.ndarray:
    row = np.concatenate(
        [
            np.tile(MASS, 3),  # massc[a*7+i] = m_i
            DCUM.reshape(21),  # dcum[a*7+j]
        ]
    ).astype(NPBF)
    return np.ascontiguousarray(
        np.broadcast_to(row[None, :, None], (P, NCST, F))
    ).reshape(P, NCST * F)


def build_nc():
    nc = bacc.Bacc("TRN2")

    _nb = nc.alloc_sbuf_tensor("const-float32-negbeta", [128, 1], mybir.dt.float32)
    nc.gpsimd.memset(_nb.ap(), -BETA)
    nc.const_aps.aps[(mybir.dt.float32, -BETA)] = _nb.ap()
    nc.all_engine_barrier()

    x_in = nc.dram_tensor("x", [NBLK, P, 63 * F], BF, kind="ExternalInput")
    cst_in = nc.dram_tensor("cst", [P, NCST * F], BF, kind="ExternalInput")
    out_d = nc.dram_tensor("out", [NBLK, P, 42 * F], BF, kind="ExternalOutput")

    V = nc.vector
    G_ = nc.gpsimd

    def emul(E, out, a, b):
        E.scalar_tensor_tensor(out, a, 1.0, b, MUL, MUL)

    def eadd(E, out, a, b):
        E.scalar_tensor_tensor(out, a, 1.0, b, MUL, ADD)

    def esub(E, out, a, b):  # out = a - b
        E.scalar_tensor_tensor(out, b, -1.0, a, MUL, ADD)

    def efma(E, out, a, s, b):  # out = s*a + b
        E.scalar_tensor_tensor(out, a, s, b, MUL, ADD)

    with tile.TileContext(nc) as tc:
        with (
            tc.tile_pool(name="cstp", bufs=1) as cstp,
            tc.tile_pool(name="io", bufs=2) as io,
            tc.tile_pool(name="wk", bufs=2) as wk,
        ):
            cst = cstp.tile([P, NCST * F], BF, tag="cst")
            nc.scalar.dma_start(cst[:], cst_in[:])
            cv = cst[:].rearrange("p (c f) -> p c f", c=NCST, f=F)
            masscv = cv[:, 0:21, :].rearrange("p (a i) f -> p a i f", a=3, i=7)
            dcumv = cv[:, 21:42, :]  # [P,21,F] flat (a j)

            def r2(t, n):  # [P, n, F]
                return t[:].rearrange("p (c f) -> p c f", c=n, f=F)

            def r3(t, a, i):  # [P, a, i, F]
                return t[:].rearrange("p (a i f) -> p a i f", a=a, i=i, f=F)

            def bj(v):  # [P,F] -> [P,7,F] broadcast over j (outermost)
                return v.unsqueeze(1).broadcast_to([P, 7, F])

            for b in range(NBLK):
                xt = io.tile([P, 63 * F], BF, tag="xt")
                nc.sync.dma_start(xt[:], x_in[b])
                xv = r3(xt, 9, 7)
                Cv = xv[:, 0:3]  # [P,3,7,F]
                Ppv = xv[:, 3:6]
                Jv = xv[:, 6:9]
                Cf = xv[:, 0:3].rearrange("p a i f -> p (a i) f")
                Ppf = xv[:, 3:6].rearrange("p a i f -> p (a i) f")

                # ---------------- DVE phase 1: rp, mc, rt
                rp = wk.tile([P, 21 * F], BF, tag="rp")
                rpv = r3(rp, 3, 7)
                rpf = r2(rp, 21)
                esub(V, rpf, Cf, Ppf)
                mc = wk.tile([P, 21 * F], BF, tag="mc")
                mcv = r3(mc, 3, 7)
                mcf = r2(mc, 21)
                emul(V, mcf, cv[:, 0:21, :], Cf)
                y9 = wk.tile([P, 9 * F], BF, tag="y9")
                y9v = r3(y9, 3, 3)
                eadd(V, y9v, mcv[:, :, 0:3, :], mcv[:, :, 3:6, :])
                rt = wk.tile([P, 3 * F], BF, tag="rt")
                rtv = r2(rt, 3)
                eadd(V, rtv, y9v[:, :, 0, :], y9v[:, :, 1, :])
                eadd(V, rtv, rtv, y9v[:, :, 2, :])
                eadd(V, rtv, rtv, mcv[:, :, 6, :])

                # ---------------- ACT: rs ; Pool: q ; ACT: g, w ; Pool: rj
                rt_b4 = rtv.unsqueeze(2).broadcast_to([P, 3, 7, F])
                rs = wk.tile([P, 3 * F], BF, tag="rs")
                rsv = r2(rs, 3)
                nc.scalar.mul(rsv[:, 0:2, :], rtv[:, 0:2, :], 1.0 / M_TOT)
                nc.scalar.activation(
                    rsv[:, 2, :],
                    rtv[:, 2, :],
                    mybir.ActivationFunctionType.Identity,
                    bias=-BETA,
                    scale=1.0 / M_TOT,
                )
                q3 = wk.tile([P, 3 * F], BF, tag="q3")
                q3v = r2(q3, 3)
                G_.tensor_mul(q3v, rsv, rsv)
                q = wk.tile([P, F], BF, tag="q")
                qv = q[:]
                G_.tensor_add(qv, q3v[:, 0, :], q3v[:, 1, :])
                G_.tensor_add(qv, qv, q3v[:, 2, :])
                g = wk.tile([P, 3 * F], BF, tag="g")
                gv = r2(g, 3)
                w = wk.tile([P, 3 * F], BF, tag="w")
                wv = r2(w, 3)
                for a in range(3):
                    kc = float(K / C1[a])
                    nc.scalar.activation(
                        gv[:, a, :],
                        qv,
                        mybir.ActivationFunctionType.Identity,
                        bias=1.0,
                        scale=kc,
                    )
                    nc.scalar.mul(wv[:, a, :], rsv[:, a, :], kc)
                rjp = wk.tile([P, 21 * F], BF, tag="rjp")
                rjpv = r3(rjp, 3, 7)
                G_.tensor_mul(rjpv, rt_b4, Jv)
                rj = wk.tile([P, 7 * F], BF, tag="rj")
                rjv = r2(rj, 7)
                G_.tensor_add(rjv, rjpv[:, 0], rjpv[:, 1])
                G_.tensor_add(rjv, rjv, rjpv[:, 2])

                # ---------------- DVE phase 2: U, suffix, T, rr, u, a1
                ut = wk.tile([P, 84 * F], BF, tag="ut")
                Uv = ut[:].rearrange(
                    "p (a d i f) -> p a d i f", a=3, d=4, i=7, f=F
                )
                for a in range(3):
                    rp_a = rpv[:, a].unsqueeze(1).broadcast_to([P, 3, 7, F])
                    emul(V, Uv[:, a, 0:3], rp_a, mcv)
                emul(V, Uv[:, :, 3], rpv, masscv)
                for j in range(5, -1, -1):
                    eadd(V, Uv[:, :, :, j, :], Uv[:, :, :, j, :], Uv[:, :, :, j + 1, :])
                gd = Uv[:, :, 0:3]  # [P,3(a),3(d),7,F]
                rsuf = Uv[:, :, 3]  # [P,3,7,F]

                trg = wk.tile([P, 7 * F], BF, tag="trg")
                trgv = r2(trg, 7)
                eadd(V, trgv, gd[:, 0, 0], gd[:, 1, 1])
                eadd(V, trgv, trgv, gd[:, 2, 2])

                tp = wk.tile([P, 63 * F], BF, tag="tp")
                tpv = tp[:].rearrange(
                    "p (a d j f) -> p a d j f", a=3, d=3, j=7, f=F
                )
                J_b = (
                    Jv.rearrange("p d j f -> p (d j) f")
                    .unsqueeze(1)
                    .broadcast_to([P, 3, 21, F])
                )
                emul(
                    V,
                    tpv.rearrange("p a d j f -> p a (d j) f"),
                    gd.rearrange("p a d j f -> p a (d j) f"),
                    J_b,
                )
                tt = wk.tile([P, 21 * F], BF, tag="tt")
                ttv = r3(tt, 3, 7)
                eadd(V, ttv, tpv[:, :, 0], tpv[:, :, 1])
                eadd(V, ttv, ttv, tpv[:, :, 2])

                rrp = wk.tile([P, 21 * F], BF, tag="rrp")
                rrpv = r3(rrp, 3, 7)
                G_.tensor_mul(rrpv, rt_b4, rsuf)
                rr = wk.tile([P, 7 * F], BF, tag="rr")
                rrv = r2(rr, 7)
                eadd(V, rrv, rrpv[:, 0], rrpv[:, 1])
                eadd(V, rrv, rrv, rrpv[:, 2])
                u7 = wk.tile([P, 7 * F], BF, tag="u7")
                u7v = r2(u7, 7)
                efma(V, u7v, rrv, -1.0 / M_TOT, trgv)
                a1 = wk.tile([P, 21 * F], BF, tag="a1")
                a13 = a1[:].rearrange("p (a x) -> p a x", a=3, x=7 * F)
                dcum3 = cst[:, 21 * F : 42 * F].rearrange(
                    "p (a x) -> p a x", a=3, x=7 * F
                )
                u_b = u7[:].unsqueeze(1).broadcast_to([P, 3, 7 * F])
                eadd(V, a13, dcum3, u_b)

                # ---------------- Pool phase B: vr, jtw
                vr = wk.tile([P, 21 * F], BF, tag="vr")
                vrv = r3(vr, 3, 7)
                vr3 = vr[:].rearrange("p (a x) -> p a x", a=3, x=7 * F)
                rj_b = rj[:].unsqueeze(1).broadcast_to([P, 3, 7 * F])
                rsuf3 = rsuf.rearrange("p a i f -> p a (i f)")
                G_.tensor_mul(vr3, rj_b, rsuf3)
                ja = wk.tile([P, 21 * F], BF, tag="ja")
                jav = r3(ja, 3, 7)
                jb = wk.tile([P, 21 * F], BF, tag="jb")
                jbv = r3(jb, 3, 7)
                for a in range(3):
                    a1_, a2_ = (a + 1) % 3, (a + 2) % 3
                    G_.tensor_mul(jav[:, a], Jv[:, a1_], rsuf[:, a2_])
                    emul(V, jbv[:, a], Jv[:, a2_], rsuf[:, a1_])
                esub(V, r2(ja, 21), r2(ja, 21), r2(jb, 21))  # jtw

                # ---------------- DVE phase 3: hth, Y, s, bot, top
                hth = wk.tile([P, 21 * F], BF, tag="hth")
                hthv = r3(hth, 3, 7)
                hthf = r2(hth, 21)
                emul(V, hthf, r2(a1, 21), xv[:, 6:9].rearrange("p a i f -> p (a i) f"))
                esub(V, hthf, hthf, r2(tt, 21))
                efma(V, hthf, r2(vr, 21), 1.0 / M_TOT, hthf)

                Y = wk.tile([P, 21 * F], BF, tag="Y")
                Yv = r3(Y, 3, 7)
                for a in range(3):
                    V.tensor_scalar_mul(
                        Yv[:, a].rearrange("p i f -> p (i f)"),
                        hthv[:, a].rearrange("p i f -> p (i f)"),
                        float(1.0 / C1[a]),
                    )
                sp = wk.tile([P, 21 * F], BF, tag="sp")
                spv = r3(sp, 3, 7)
                for a in range(3):
                    emul(V, spv[:, a], bj(rsv[:, a, :]), Yv[:, a])
                s7 = wk.tile([P, 7 * F], BF, tag="s7")
                s7v = r2(s7, 7)
                eadd(V, s7v, spv[:, 0], spv[:, 1])
                eadd(V, s7v, s7v, spv[:, 2])

                outt = io.tile([P, 42 * F], BF, tag="outt")
                outv = r3(outt, 6, 7)

                gy = wk.tile([P, 21 * F], BF, tag="gy")
                gyv = r3(gy, 3, 7)
                t1 = wk.tile([P, 21 * F], BF, tag="t1")
                t1v = r3(t1, 3, 7)
                for a in range(3):
                    emul(V, gyv[:, a], bj(gv[:, a, :]), Yv[:, a])
                    emul(V, t1v[:, a], bj(wv[:, a, :]), s7v)
                esub(
                    V,
                    outv[:, 3:6].rearrange("p a j f -> p (a j) f"),
                    r2(t1, 21),
                    r2(gy, 21),
                )  # bot

                ctb = wk.tile([P, 21 * F], BF, tag="ctb")
                ctbv = r3(ctb, 3, 7)
                ctc = wk.tile([P, 21 * F], BF, tag="ctc")
                ctcv = r3(ctc, 3, 7)
                for a in range(3):
                    a1_, a2_ = (a + 1) % 3, (a + 2) % 3
                    emul(V, ctbv[:, a], bj(rsv[:, a1_, :]), outv[:, 3 + a2_])
                    emul(V, ctcv[:, a], bj(rsv[:, a2_, :]), outv[:, 3 + a1_])
                esub(V, r2(ctb, 21), r2(ctb, 21), r2(ctc, 21))
                efma(
                    V,
                    outv[:, 0:3].rearrange("p a j f -> p (a j) f"),
                    r2(ja, 21),
                    -1.0 / M_TOT,
                    r2(ctb, 21),
                )  # top

                nc.scalar.dma_start(out_d[b], outt[:])

    nc.compile()
    return nc


_NC_CACHE = None


def _get_nc():
    global _NC_CACHE
    if _NC_CACHE is None:
        _NC_CACHE = build_nc()
    return _NC_CACHE


def _shard_inputs(com_list, link_pose_list, jacobian):
    S = N_SAMPLES * N_HORIZON
    com = np.asarray(com_list, np.float32).reshape(S, 21)
    pos = np.ascontiguousarray(
        np.asarray(link_pose_list, np.float32).reshape(S, 4, 4, 9)[:, 0:3, 3, 0:7]
    ).reshape(S, 21)
    j3 = np.ascontiguousarray(
        np.asarray(jacobian, np.float32).reshape(S, 6, 7)[:, 0:3, :]
    ).reshape(S, 21)
    x = np.concatenate([com, pos, j3], axis=1).astype(NPBF)  # (S, 63)
    x = np.ascontiguousarray(
        x.reshape(N_CORES, NBLK, P, F, 63).transpose(0, 1, 2, 4, 3)
    )  # (cores, NBLK, P, 63, F)
    cst = _const_array()
    return [
        {"x": x[c].reshape(NBLK, P, 63 * F), "cst": cst} for c in range(N_CORES)
    ]


def _gather(results):
    outs = np.stack([r["out"] for r in results])  # (8, NBLK, P, 42F) bf16
    o = outs.reshape(N_CORES, NBLK, P, 42, F).transpose(0, 1, 2, 4, 3)
    return np.ascontiguousarray(o).astype(np.float32).reshape(
        N_SAMPLES, N_HORIZON, 6, 7
    )


def run(com_list, link_pose_list, jacobian, trace=False):
    nc = _get_nc()
    in_maps = _shard_inputs(com_list, link_pose_list, jacobian)
    res = run_bass_kernel_spmd(nc, in_maps, list(range(N_CORES)), trace=trace)
    return _gather(res.results), res


def kernel(com_list, link_pose_list, jacobian):
    out, _ = run(com_list, link_pose_list, jacobian)
    return out


# revision 11
# speedup vs baseline: 1.6523x; 1.6249x over previous
"""Trainium2 Bass kernel for nn_CanadarmJacob (centroidal-dynamics jacobian).

Data-parallel over 8 NeuronCores; per core 32768 flat samples split into
NBLK=4 blocks of [P=128 partitions, F=64 free].  All per-sample quantities
live channel-major ([P, ch*F]) so every vector-op operand has a unit-stride
F-sized last dim -> DVE 2-byte fast modes apply.  Whole pipeline is bf16
scalar_tensor_tensor / tensor_scalar (InstTensorScalarPtr: 4x on DVE,
0.60-eff on Pool); ops are shaped so every access pattern canonicalizes to
<= 3 dims (stt verifier limit).  Only the 63 input floats/sample the
reference actually reads are shipped (com 21, link positions 21, jacobian
rows 0:3), packed host-side into one fused bf16 tensor.

Math (same validated algebra as the fp32 baseline):
  RP = C - P ;  MC = m_i*C ;  U[a,dd,i] = RP[a]*(MC[dd] | m_i)
  G = suffix_j(U) -> G[a,d,j], R[a,j] ;  rt = sum_i MC
  r = rt/M_tot - (0,0,beta) ;  T[a,j] = sum_d G[a,d,j]*J[d,j]
  trG ; rr = sum_a rt[a]R[a,j] ; rj = sum_a rt[a]J[a,j]
  u = trG - rr/M_tot ;  H_th = (DCUM + u)*J - T + (rj_b*R)/M_tot
  J_tw = J_j x R_j
H_s = K r r^T + diag(C1 - K|r|^2) = D + P_m with D = diag(C1) constant and
|P_m|/|D| <= ~4e-3, so first-order Neumann:  H_s^-1 ~= D^-1 - D^-1 P_m D^-1
  Y = Hth/C1 ;  s = sum_a r[a]Y[a,:] ;  q = |r|^2
  bot = w[a]*s - g[a]*Y ,  g = 1 + (K/C1[a]) q ,  w = (K/C1[a]) r[a]
  top = -J_tw/M_tot + r x bot
"""

import os
import sys

for _p in ("/opt/trn_rl_repo", "/root/.axon_site/_ro/trn_rl_repo"):
    if os.path.isdir(_p) and _p not in sys.path:
        sys.path.append(_p)

import numpy as np
import ml_dtypes

import concourse.bass as bass
import concourse.tile as tile
from concourse import bacc, mybir
from concourse.bass_utils import run_bass_kernel_spmd

# ----------------------------------------------------------------- constants
N_SAMPLES, N_HORIZON = 2048, 128
N_CORES = 8
P = 128
F = 64
SPC = N_SAMPLES // N_CORES * N_HORIZON  # 32768
NBLK = SPC // (P * F)  # 4

BASE_MASS, EEF_MASS = 100000.0, 243.66
MASS = np.array([105.98, 105.98, 314.98, 279.2, 105.98, 105.98, 243.66], np.float32)
DIAGS = np.array(
    [
        [12.19, 12.19, 3.061],
        [12.19, 12.19, 3.061],
        [15.41, 2094.71, 2103.19],
        [9.522, 1966.28, 1966.28],
        [8.305, 3.061, 8.0386],
        [12.13, 12.13, 3.061],
        [9.336, 44.41, 44.41],
    ],
    np.float32,
)
I0DIAG = np.array([69585.02, 69585.02, 66666.664], np.float32)

M_MAN = float(MASS.sum())
M_TOT = M_MAN + BASE_MASS + EEF_MASS
K = BASE_MASS + EEF_MASS
BETA = 6.65 * (243.66 / (100000.0 + 243.66))
DCUM = np.stack([DIAGS[j:].sum(0) for j in range(7)], axis=1)  # [a][j]
C1 = (DIAGS.sum(0) + I0DIAG).astype(np.float64)  # [a]

BF = mybir.dt.bfloat16
NPBF = ml_dtypes.bfloat16
ADD = mybir.AluOpType.add
MUL = mybir.AluOpType.mult

NCST = 42  # massc 21 | dcum 21


def _const_array() -> np.ndarray:
    row = np.concatenate(
        [
            np.tile(MASS, 3),  # massc[a*7+i] = m_i
            DCUM.reshape(21),  # dcum[a*7+j]
        ]
    ).astype(NPBF)
    return np.ascontiguousarray(
        np.broadcast_to(row[None, :, None], (P, NCST, F))
    ).reshape(P, NCST * F)


def build_nc():
    nc = bacc.Bacc("TRN2")

    _nb = nc.alloc_sbuf_tensor("const-float32-negbeta", [128, 1], mybir.dt.float32)
    nc.gpsimd.memset(_nb.ap(), -BETA)
    nc.const_aps.aps[(mybir.dt.float32, -BETA)] = _nb.ap()
    nc.all_engine_barrier()

    x_in = nc.dram_tensor("x", [NBLK, P, 63 * F], BF, kind="ExternalInput")
    cst_in = nc.dram_tensor("cst", [P, NCST * F], BF, kind="ExternalInput")
    out_d = nc.dram_tensor("out", [NBLK, P, 42 * F], BF, kind="ExternalOutput")

    V = nc.vector
    G_ = nc.gpsimd

    def emul(E, out, a, b):
        E.scalar_tensor_tensor(out, a, 1.0, b, MUL, MUL)

    def eadd(E, out, a, b):
        E.scalar_tensor_tensor(out, a, 1.0, b, MUL, ADD)

    def esub(E, out, a, b):  # out = a - b
        E.scalar_tensor_tensor(out, b, -1.0, a, MUL, ADD)

    def efma(E, out, a, s, b):  # out = s*a + b
        E.scalar_tensor_tensor(out, a, s, b, MUL, ADD)

    with tile.TileContext(nc) as tc:
        with (
            tc.tile_pool(name="cstp", bufs=1) as cstp,
            tc.tile_pool(name="io", bufs=2) as io,
            tc.tile_pool(name="wk", bufs=2) as wk,
        ):
            cst = cstp.tile([P, NCST * F], BF, tag="cst")
            nc.scalar.dma_start(cst[:], cst_in[:])
            cv = cst[:].rearrange("p (c f) -> p c f", c=NCST, f=F)
            masscv = cv[:, 0:21, :].rearrange("p (a i) f -> p a i f", a=3, i=7)
            dcumv = cv[:, 21:42, :]  # [P,21,F] flat (a j)

            def r2(t, n):  # [P, n, F]
                return t[:].rearrange("p (c f) -> p c f", c=n, f=F)

            def r3(t, a, i):  # [P, a, i, F]
                return t[:].rearrange("p (a i f) -> p a i f", a=a, i=i, f=F)

            def bj(v):  # [P,F] -> [P,7,F] broadcast over j (outermost)
                return v.unsqueeze(1).broadcast_to([P, 7, F])

            for b in range(NBLK):
                xt = io.tile([P, 63 * F], BF, tag="xt")
                nc.sync.dma_start(xt[:], x_in[b])
                xv = r3(xt, 9, 7)
                Cv = xv[:, 0:3]  # [P,3,7,F]
                Ppv = xv[:, 3:6]
                Jv = xv[:, 6:9]

                # ---------------- DVE: rp, mc ; Pool: rt tree
                rp = wk.tile([P, 21 * F], BF, tag="rp")
                rpv = r3(rp, 3, 7)
                V.tensor_sub(rpv, Cv, Ppv)
                mc = wk.tile([P, 21 * F], BF, tag="mc")
                mcv = r3(mc, 3, 7)
                V.tensor_mul(mcv, masscv, Cv)
                y9 = wk.tile([P, 9 * F], BF, tag="y9")
                y9v = r3(y9, 3, 3)
                G_.tensor_add(y9v, mcv[:, :, 0:3, :], mcv[:, :, 3:6, :])
                rt = wk.tile([P, 3 * F], BF, tag="rt")
                rtv = r2(rt, 3)
                G_.tensor_add(rtv, y9v[:, :, 0, :], y9v[:, :, 1, :])
                G_.tensor_add(rtv, rtv, y9v[:, :, 2, :])
                G_.tensor_add(rtv, rtv, mcv[:, :, 6, :])
                rt_b4 = rtv.unsqueeze(2).broadcast_to([P, 3, 7, F])

                # ---------------- ACT: rs, w ; Pool: q ; ACT: g
                rs = wk.tile([P, 3 * F], BF, tag="rs")
                rsv = r2(rs, 3)
                nc.scalar.mul(rsv[:, 0:2, :], rtv[:, 0:2, :], 1.0 / M_TOT)
                nc.scalar.activation(
                    rsv[:, 2, :],
                    rtv[:, 2, :],
                    mybir.ActivationFunctionType.Identity,
                    bias=-BETA,
                    scale=1.0 / M_TOT,
                )
                w = wk.tile([P, 3 * F], BF, tag="w")
                wv = r2(w, 3)
                q3 = wk.tile([P, 3 * F], BF, tag="q3")
                q3v = r2(q3, 3)
                G_.tensor_mul(q3v, rsv, rsv)
                q = wk.tile([P, F], BF, tag="q")
                qv = q[:]
                G_.tensor_add(qv, q3v[:, 0, :], q3v[:, 1, :])
                G_.tensor_add(qv, qv, q3v[:, 2, :])
                g = wk.tile([P, 3 * F], BF, tag="g")
                gv = r2(g, 3)
                for a in range(3):
                    kc = float(K / C1[a])
                    nc.scalar.activation(
                        gv[:, a, :],
                        qv,
                        mybir.ActivationFunctionType.Identity,
                        bias=1.0,
                        scale=kc,
                    )
                    nc.scalar.mul(wv[:, a, :], rsv[:, a, :], kc)

                # ---------------- DVE: scaled copies ; Pool: rj
                rtm = wk.tile([P, 3 * F], BF, tag="rtm")
                rtmv = r2(rtm, 3)
                V.tensor_scalar_mul(rtmv, rtv, -1.0 / M_TOT)
                jm = wk.tile([P, 21 * F], BF, tag="jm")
                jmv = r3(jm, 3, 7)
                V.tensor_scalar_mul(r2(jm, 21), xv[:, 6:9].rearrange("p a i f -> p (a i) f"), -1.0 / M_TOT)
                rjp = wk.tile([P, 21 * F], BF, tag="rjp")
                rjpv = r3(rjp, 3, 7)
                G_.tensor_mul(rjpv, rt_b4, Jv)
                rj = wk.tile([P, 7 * F], BF, tag="rj")
                rjv = r2(rj, 7)
                G_.tensor_add(rjv, rjpv[:, 0], rjpv[:, 1])
                G_.tensor_add(rjv, rjv, rjpv[:, 2])
                rjm = wk.tile([P, 7 * F], BF, tag="rjm")
                rjmv = r2(rjm, 7)
                V.tensor_scalar_mul(rjmv, rjv, 1.0 / M_TOT)

                # ---------------- DVE: U, suffix, trg, T
                ut = wk.tile([P, 84 * F], BF, tag="ut")
                Uv = ut[:].rearrange(
                    "p (a d i f) -> p a d i f", a=3, d=4, i=7, f=F
                )
                for a in range(3):
                    rp_a = rpv[:, a].unsqueeze(1).broadcast_to([P, 3, 7, F])
                    V.tensor_mul(Uv[:, a, 0:3], rp_a, mcv)
                V.tensor_mul(Uv[:, :, 3], rpv, masscv)
                for j in range(5, -1, -1):
                    V.tensor_add(
                        Uv[:, :, :, j, :], Uv[:, :, :, j, :], Uv[:, :, :, j + 1, :]
                    )
                gd = Uv[:, :, 0:3]  # [P,3(a),3(d),7,F]
                rsuf = Uv[:, :, 3]  # [P,3,7,F]
                rsuf3 = rsuf.rearrange("p a i f -> p a (i f)")

                trg = wk.tile([P, 7 * F], BF, tag="trg")
                trgv = r2(trg, 7)
                V.tensor_add(trgv, gd[:, 0, 0], gd[:, 1, 1])
                V.tensor_add(trgv, trgv, gd[:, 2, 2])

                tp = wk.tile([P, 63 * F], BF, tag="tp")
                tpv = tp[:].rearrange(
                    "p (a d j f) -> p a d j f", a=3, d=3, j=7, f=F
                )
                J_b = (
                    Jv.rearrange("p d j f -> p (d j) f")
                    .unsqueeze(1)
                    .broadcast_to([P, 3, 21, F])
                )
                V.tensor_mul(tpv.rearrange("p a d j f -> p a (d j) f"),
                             gd.rearrange("p a d j f -> p a (d j) f"), J_b)
                tt = wk.tile([P, 21 * F], BF, tag="tt")
                ttv = r3(tt, 3, 7)
                V.tensor_add(ttv, tpv[:, :, 0], tpv[:, :, 1])
                V.tensor_add(ttv, ttv, tpv[:, :, 2])

                # ---------------- Pool: rrp', vr', ja/jb/jtw'
                rtm_b4 = rtmv.unsqueeze(2).broadcast_to([P, 3, 7, F])
                rrp = wk.tile([P, 21 * F], BF, tag="rrp")
                rrpv = r3(rrp, 3, 7)
                G_.tensor_mul(rrpv, rtm_b4, rsuf)
                vr = wk.tile([P, 21 * F], BF, tag="vr")
                vr3 = vr[:].rearrange("p (a x) -> p a x", a=3, x=7 * F)
                rjm_b = rjm[:].unsqueeze(1).broadcast_to([P, 3, 7 * F])
                G_.tensor_mul(vr3, rjm_b, rsuf3)
                ja = wk.tile([P, 21 * F], BF, tag="ja")
                jav = r3(ja, 3, 7)
                jb = wk.tile([P, 21 * F], BF, tag="jb")
                jbv = r3(jb, 3, 7)
                for a in range(3):
                    a1_, a2_ = (a + 1) % 3, (a + 2) % 3
                    G_.tensor_mul(jav[:, a], jmv[:, a1_], rsuf[:, a2_])
                    G_.tensor_mul(jbv[:, a], jmv[:, a2_], rsuf[:, a1_])
                G_.tensor_sub(r2(ja, 21), r2(ja, 21), r2(jb, 21))  # jtw' = -J_tw/M

                # ---------------- DVE: rr', u, a1, hth, Y, s, bot, top
                rr = wk.tile([P, 7 * F], BF, tag="rr")
                rrv = r2(rr, 7)
                V.tensor_add(rrv, rrpv[:, 0], rrpv[:, 1])
                V.tensor_add(rrv, rrv, rrpv[:, 2])
                u7 = wk.tile([P, 7 * F], BF, tag="u7")
                u7v = r2(u7, 7)
                V.tensor_add(u7v, trgv, rrv)  # u = trg - rr/M
                a1 = wk.tile([P, 21 * F], BF, tag="a1")
                a13 = a1[:].rearrange("p (a x) -> p a x", a=3, x=7 * F)
                dcum3 = cst[:, 21 * F : 42 * F].rearrange(
                    "p (a x) -> p a x", a=3, x=7 * F
                )
                u_b = u7[:].unsqueeze(1).broadcast_to([P, 3, 7 * F])
                V.tensor_add(a13, dcum3, u_b)

                hth = wk.tile([P, 21 * F], BF, tag="hth")
                hthv = r3(hth, 3, 7)
                hthf = r2(hth, 21)
                V.tensor_mul(hthf, r2(a1, 21),
                             xv[:, 6:9].rearrange("p a i f -> p (a i) f"))
                V.tensor_sub(hthf, hthf, r2(tt, 21))
                V.tensor_add(hthf, hthf, r2(vr, 21))

                Y = wk.tile([P, 21 * F], BF, tag="Y")
                Yv = r3(Y, 3, 7)
                for a in range(3):
                    V.tensor_scalar_mul(
                        Yv[:, a].rearrange("p i f -> p (i f)"),
                        hthv[:, a].rearrange("p i f -> p (i f)"),
                        float(1.0 / C1[a]),
                    )
                sp = wk.tile([P, 21 * F], BF, tag="sp")
                spv = r3(sp, 3, 7)
                for a in range(3):
                    V.tensor_mul(spv[:, a], bj(rsv[:, a, :]), Yv[:, a])
                s7 = wk.tile([P, 7 * F], BF, tag="s7")
                s7v = r2(s7, 7)
                V.tensor_add(s7v, spv[:, 0], spv[:, 1])
                V.tensor_add(s7v, s7v, spv[:, 2])

                outt = io.tile([P, 42 * F], BF, tag="outt")
                outv = r3(outt, 6, 7)

                gy = wk.tile([P, 21 * F], BF, tag="gy")
                gyv = r3(gy, 3, 7)
                t1 = wk.tile([P, 21 * F], BF, tag="t1")
                t1v = r3(t1, 3, 7)
                for a in range(3):
                    V.tensor_mul(gyv[:, a], bj(gv[:, a, :]), Yv[:, a])
                    V.tensor_mul(t1v[:, a], bj(wv[:, a, :]), s7v)
                V.tensor_sub(
                    outv[:, 3:6].rearrange("p a j f -> p (a j) f"),
                    r2(t1, 21),
                    r2(gy, 21),
                )  # bot

                ctb = wk.tile([P, 21 * F], BF, tag="ctb")
                ctbv = r3(ctb, 3, 7)
                ctc = wk.tile([P, 21 * F], BF, tag="ctc")
                ctcv = r3(ctc, 3, 7)
                for a in range(3):
                    a1_, a2_ = (a + 1) % 3, (a + 2) % 3
                    V.tensor_mul(ctbv[:, a], bj(rsv[:, a1_, :]), outv[:, 3 + a2_])
                    V.tensor_mul(ctcv[:, a], bj(rsv[:, a2_, :]), outv[:, 3 + a1_])
                V.tensor_sub(r2(ctb, 21), r2(ctb, 21), r2(ctc, 21))
                V.tensor_add(
                    outv[:, 0:3].rearrange("p a j f -> p (a j) f"),
                    r2(ja, 21),
                    r2(ctb, 21),
                )  # top = jtw' + r x bot

                nc.scalar.dma_start(out_d[b], outt[:])

    nc.compile()
    return nc


_NC_CACHE = None


def _get_nc():
    global _NC_CACHE
    if _NC_CACHE is None:
        _NC_CACHE = build_nc()
    return _NC_CACHE


def _shard_inputs(com_list, link_pose_list, jacobian):
    S = N_SAMPLES * N_HORIZON
    com = np.asarray(com_list, np.float32).reshape(S, 21)
    pos = np.ascontiguousarray(
        np.asarray(link_pose_list, np.float32).reshape(S, 4, 4, 9)[:, 0:3, 3, 0:7]
    ).reshape(S, 21)
    j3 = np.ascontiguousarray(
        np.asarray(jacobian, np.float32).reshape(S, 6, 7)[:, 0:3, :]
    ).reshape(S, 21)
    x = np.concatenate([com, pos, j3], axis=1).astype(NPBF)  # (S, 63)
    x = np.ascontiguousarray(
        x.reshape(N_CORES, NBLK, P, F, 63).transpose(0, 1, 2, 4, 3)
    )  # (cores, NBLK, P, 63, F)
    cst = _const_array()
    return [
        {"x": x[c].reshape(NBLK, P, 63 * F), "cst": cst} for c in range(N_CORES)
    ]


def _gather(results):
    outs = np.stack([r["out"] for r in results])  # (8, NBLK, P, 42F) bf16
    o = outs.reshape(N_CORES, NBLK, P, 42, F).transpose(0, 1, 2, 4, 3)
    return np.ascontiguousarray(o).astype(np.float32).reshape(
        N_SAMPLES, N_HORIZON, 6, 7
    )


def run(com_list, link_pose_list, jacobian, trace=False):
    nc = _get_nc()
    in_maps = _shard_inputs(com_list, link_pose_list, jacobian)
    res = run_bass_kernel_spmd(nc, in_maps, list(range(N_CORES)), trace=trace)
    return _gather(res.results), res


def kernel(com_list, link_pose_list, jacobian):
    out, _ = run(com_list, link_pose_list, jacobian)
    return out
